# revision 21
# baseline (speedup 1.0000x reference)
"""KimiMoEGate (sigmoid scoring, group-limited top-k) on 8 Trainium2 cores.

Strategy (hardcoded for hidden_states [4,4096,2048], weight [256,2048]):
  - Token-parallel: 16384 tokens sharded 2048/core across 8 cores; router
    weight + bias replicated per core.
  - Router logits at ~fp32 accuracy in 2.0 fp16-pass-equivalents on the PE:
      main:  xh(fp16) @ wh(fp16)          -> P0   (16 matmuls, 1 cyc/row)
      corrA: x8(fp8)  @ wl8(fp8)  2^-16   -> Pc   (8 DoubleRow matmuls)
      corrB: xl8(fp8) @ wh8(fp8)  2^-16   -> Pc   (8 DoubleRow matmuls)
    where xh=fp16(x), xl=x-xh, wh=fp16(1024W), wl=1024W-wh, and the fp8
    operands carry power-of-2 scales (x8: 2^5, wl8: 2^11, xl8: 2^16,
    wh8: 2^0) so both corrections land in one PSUM at scale 2^16.
  - ACT applies sigmoid (scale 2^-10 folds away the 1024). Group-limited
    top-k via DVE max8 / max_index / match_replace on exact fp32 biased
    scores.
  - Bit-packing instead of a per-expert gather: the ranking tensor is
    rne(sfc*2^19)/2^19 + pbq[e]*2^-24 with pbq the expert bias quantized to
    4 bits. The top-8 values then carry their own bias: a +24/-24 magic
    round-trip splits them back into quantized score and packed bias, so
    w_j needs no gather.  Scale/bias-only pack steps run on the ACT engine.
  - Pipeline: tiny PE warm-up matmuls start the ramp clock; router weights
    stream in 4 chunked DMAs behind the first x tile; per-tile weight
    normalization and split output DMAs shorten the tail.
"""

import numpy as np
import ml_dtypes

from concourse import bacc, bass_utils
import concourse.mybir as mybir
from concourse.tile import TileContext

F16 = mybir.dt.float16
F32 = mybir.dt.float32
F8 = mybir.dt.float8e4
U16 = mybir.dt.uint16
AF = mybir.ActivationFunctionType
ALU = mybir.AluOpType
AX = mybir.AxisListType
NPF8 = ml_dtypes.float8_e4m3

N_CORES = 8
N_GROUP = 8
EXP_PER_GROUP = 32
E = 256
H = 2048
H_CHUNKS = 16  # 2048 / 128
T_TOTAL = 16384
T_CORE = T_TOTAL // N_CORES
N_TILES = T_CORE // 128  # 16
N_WARM = 6

MAGIC = float(1.5 * 2.0 ** 23)  # integer-rounding magic at the 2^19 scale
QOFF = float(1.5 * 2.0 ** 4)    # same magic at the v0 scale: rounds to 2^-19

SX8 = 5    # x8  = fp8(x * 2^5)
SWL = 11   # wl8 = fp8(wl * 2^11)
SXL = 16   # xl8 = fp8(xl * 2^16)
SWH = 0    # wh8 = fp8(wh)
SCORR = float(2.0 ** -(SX8 + SWL))  # = 2^-16 = 2^-(SXL+SWH)


def build_kernel(nc, n_tiles=N_TILES):
    xh = nc.dram_tensor("xh", [n_tiles, 128, H_CHUNKS, 128], F16, kind="ExternalInput").ap()
    # x88[..., 0:128] = x8 chunk (x * 2^5), x88[..., 128:256] = xl8 chunk (xl * 2^16)
    x88 = nc.dram_tensor("x88", [n_tiles, 128, H_CHUNKS, 256], F8, kind="ExternalInput").ap()
    wh16 = nc.dram_tensor("wh16", [128, H_CHUNKS, E], F16, kind="ExternalInput").ap()
    # w88[:, ho, 0:256] = wl8 chunk (wl * 2^11), w88[:, ho, 256:512] = wh8 chunk
    w88 = nc.dram_tensor("w88", [128, H_CHUNKS, 2 * E], F8, kind="ExternalInput").ap()
    bias = nc.dram_tensor("bias_rep", [128, E], F32, kind="ExternalInput").ap()
    pbt = nc.dram_tensor("pb_rep", [128, E], F32, kind="ExternalInput").ap()
    rec = nc.dram_tensor("rec_const", [128, 2], F32, kind="ExternalInput").ap()
    idx_out = nc.dram_tensor("idx_out", [n_tiles, 128, 8], U16, kind="ExternalOutput").ap()
    wt_out = nc.dram_tensor("wt_out", [n_tiles, 128, 8], F32, kind="ExternalOutput").ap()

    with TileContext(nc) as tc:
        with (
            tc.tile_pool(name="const", bufs=1) as cpool,
            tc.tile_pool(name="xin", bufs=6) as xpool,
            tc.tile_pool(name="work", bufs=6) as wpool,
            tc.tile_pool(name="psum", bufs=4, space="PSUM") as ppool,
            tc.tile_pool(name="cpsum", bufs=3, space="PSUM") as cppool,
            tc.tile_pool(name="warmps", bufs=1, space="PSUM") as wppool,
            tc.tile_pool(name="persist", bufs=1) as perspool,
        ):
            # --- PE warm-up: tiny dummy matmuls with no DMA dependency;
            # they start the PE ramp clock before the first DMAs land.
            dummy = cpool.tile([128, 64], F16)
            nc.gpsimd.memset(dummy, 0.0)
            warm_ps = wppool.tile([128, 64], F32)
            for _ in range(N_WARM):
                nc.tensor.matmul(warm_ps[0:1, :], dummy[:, 0:1], dummy,
                                 start=True, stop=True)

            wh_sb = cpool.tile([128, H_CHUNKS, E], F16)
            w88_sb = cpool.tile([128, H_CHUNKS, 2 * E], F8)
            bias_in = cpool.tile([128, E], F32)
            bias_sb = cpool.tile([128, E], F32)
            pb_in = cpool.tile([128, E], F32)
            pb_sb = cpool.tile([128, E], F32)
            rec_in = cpool.tile([128, 2], F32)
            rec_sb = cpool.tile([128, 2], F32)

            idx_u16 = perspool.tile([128, n_tiles, 8], U16)
            wt_all = perspool.tile([128, n_tiles, 8], F32)

            xtiles = {}

            def fetch(i):
                if i >= n_tiles:
                    return
                a = xpool.tile([128, H_CHUNKS, 128], F16, tag="xh")
                b = xpool.tile([128, H_CHUNKS, 256], F8, tag="x88")
                nc.sync.dma_start(a, xh[i])
                nc.sync.dma_start(b, x88[i])
                xtiles[i] = (a, b)

            # head order: xh0 + main weights first (main pass can start),
            # fp8 payloads next, consts last.
            xh0 = xpool.tile([128, H_CHUNKS, 128], F16, tag="xh")
            x880 = xpool.tile([128, H_CHUNKS, 256], F8, tag="x88")
            nc.sync.dma_start(xh0, xh[0])
            for q in range(4):
                sl = slice(4 * q, 4 * q + 4)
                nc.sync.dma_start(wh_sb[:, sl, :], wh16[:, sl, :])
            nc.sync.dma_start(x880, x88[0])
            for q in range(2):
                sl = slice(8 * q, 8 * q + 8)
                nc.sync.dma_start(w88_sb[:, sl, :], w88[:, sl, :])
            xtiles[0] = (xh0, x880)
            nc.sync.dma_start(bias_in, bias)
            nc.sync.dma_start(pb_in, pbt)
            nc.sync.dma_start(rec_in, rec)
            fetch(1)
            # engine-local copies so in-loop consumers depend on same-engine
            # producers (program order) instead of carrying DMA-sem waits.
            nc.gpsimd.tensor_copy(bias_sb, bias_in)
            nc.gpsimd.tensor_copy(pb_sb, pb_in)
            nc.vector.tensor_copy(rec_sb, rec_in)

            def stage_b(i, sfc):
                sfcg = sfc.rearrange("p (g e) -> p g e", g=N_GROUP)

                # packed ranking tensor: vq = rne(sfc*2^19)*2^-19 + pb
                # (scale/bias steps on ACT)
                mt = wpool.tile([128, E], F32, tag="mt")
                nc.scalar.activation(mt, sfc, AF.Copy, bias=MAGIC,
                                     scale=float(2.0 ** 19))
                v0 = wpool.tile([128, E], F32, tag="v0")
                nc.scalar.activation(v0, mt, AF.Copy, bias=-QOFF,
                                     scale=float(2.0 ** -19))
                vq = wpool.tile([128, E], F32, tag="vq")
                nc.vector.tensor_add(vq, v0, pb_sb)

                # group stage on exact sfc (DVE): top-2 per group of 32
                g1 = wpool.tile([128, N_GROUP], F32, tag="g1")
                nc.vector.reduce_max(g1, sfcg, axis=AX.X)
                kn = wpool.tile([128, E], F32, tag="kn")
                nc.vector.match_replace(out=kn, in_to_replace=g1, in_values=sfc,
                                        imm_value=-1e30)
                g2 = wpool.tile([128, N_GROUP], F32, tag="g2")
                nc.vector.reduce_max(g2, kn.rearrange("p (g e) -> p g e", g=N_GROUP),
                                     axis=AX.X)
                gs = wpool.tile([128, N_GROUP], F32, tag="gs")
                nc.vector.tensor_add(gs, g1, g2)

                # top-4 groups: threshold at 4th largest of the 8 group scores
                g8 = wpool.tile([128, 8], F32, tag="g8")
                nc.vector.max(out=g8, in_=gs)
                gm = wpool.tile([128, N_GROUP], F32, tag="gm")
                nc.vector.tensor_scalar(gm, gs, g8[:, 3:4], None, op0=ALU.is_ge)

                # mask the packed scores and take top-8
                tmp = wpool.tile([128, N_GROUP, EXP_PER_GROUP], F32, tag="tmp")
                nc.gpsimd.tensor_mul(tmp, vq.rearrange("p (g e) -> p g e", g=N_GROUP),
                                     gm.unsqueeze(2).to_broadcast([128, N_GROUP, EXP_PER_GROUP]))
                tmpf = tmp.rearrange("p g e -> p (g e)")
                m8 = wpool.tile([128, 8], F32, tag="m8")
                nc.vector.max(out=m8, in_=tmpf)
                nc.vector.max_index(idx_u16[:, i, :], m8, tmpf)

                # unpack (ACT magic round-trip): q8 = quantized sfc
                t1 = wpool.tile([128, 8], F32, tag="t1")
                nc.vector.tensor_scalar_add(t1, m8, QOFF)
                q8 = wpool.tile([128, 8], F32, tag="q8")
                nc.vector.tensor_scalar_sub(q8, t1, QOFF)
                pbv = wpool.tile([128, 8], F32, tag="pbv")
                nc.vector.tensor_sub(pbv, m8, q8)
                # w = q8 - (pbv * 2^24 * step + bmin)
                #   = (pbv * rec0 + q8) - rec1  [rec0 = -2^24*step, rec1 = bmin]
                u2 = wpool.tile([128, 8], F32, tag="u2")
                nc.vector.scalar_tensor_tensor(out=u2, in0=pbv,
                                               scalar=rec_sb[:, 0:1], in1=q8,
                                               op0=ALU.mult, op1=ALU.add)
                wr = wpool.tile([128, 8], F32, tag="wr")
                nc.vector.tensor_scalar(wr, u2, rec_sb[:, 1:2], None,
                                        op0=ALU.subtract)
                s = wpool.tile([128, 1], F32, tag="s")
                nc.vector.reduce_sum(s, wr.rearrange("p (o k) -> p o k", o=1),
                                     axis=AX.X)
                r = wpool.tile([128, 1], F32, tag="r")
                nc.vector.reciprocal(r, s)
                nc.vector.tensor_scalar(wt_all[:, i, :], wr, r[:, 0:1], 2.5,
                                        op0=ALU.mult, op1=ALU.mult)

            prev = None
            for i in range(n_tiles):
                fetch(i + 2)
                xh_sb, x88_sb = xtiles.pop(i)

                # main pass: P0 = xh @ wh  (1024 * logit, fp16 operands)
                p0 = ppool.tile([128, E], F32)
                for ho in range(H_CHUNKS):
                    nc.tensor.matmul(p0, xh_sb[:, ho, :], wh_sb[:, ho, :],
                                     start=(ho == 0), stop=(ho == H_CHUNKS - 1))
                # corrections: Pc = x8 @ wl8 + xl8 @ wh8 (DoubleRow fp8,
                # both at scale 2^16 relative to P0)
                pc = cppool.tile([128, E], F32)
                for hp in range(H_CHUNKS // 2):
                    sl = slice(2 * hp, 2 * hp + 2)
                    nc.tensor.matmul(pc, x88_sb[:, sl, 0:128], w88_sb[:, sl, 0:E],
                                     start=(hp == 0), stop=False,
                                     perf_mode=mybir.MatmulPerfMode.DoubleRow)
                for hp in range(H_CHUNKS // 2):
                    sl = slice(2 * hp, 2 * hp + 2)
                    nc.tensor.matmul(pc, x88_sb[:, sl, 128:256], w88_sb[:, sl, E:],
                                     start=False, stop=(hp == H_CHUNKS // 2 - 1),
                                     perf_mode=mybir.MatmulPerfMode.DoubleRow)

                # stage A: u = P0 + Pc*2^-16; scores = sigmoid(u*2^-10);
                # sfc = scores + bias.  Emitted ahead of the previous tile's
                # post-chain so the ACT->DVE->ACT ring never stalls the
                # next tile's sigmoid.
                u1 = wpool.tile([128, E], F32, tag="u1")
                nc.scalar.activation(u1, pc, AF.Copy, scale=SCORR)
                u = wpool.tile([128, E], F32, tag="u")
                nc.vector.tensor_add(u, u1, p0)
                scores = wpool.tile([128, E], F32, tag="scores")
                nc.scalar.activation(scores, u, AF.Sigmoid, scale=float(2.0 ** -10))
                sfc = wpool.tile([128, E], F32, tag="sfc")
                nc.gpsimd.tensor_add(sfc, scores, bias_sb)

                if prev is not None:
                    stage_b(*prev)
                prev = (i, sfc)

                if i == 13:
                    # input prefetch is done (fetch(15) just issued); SP is
                    # free from here, so these waits block nothing.
                    nc.sync.dma_start(idx_out[:12].rearrange("t p k -> p t k"),
                                      idx_u16[:, :12, :])
                    nc.sync.dma_start(wt_out[:12].rearrange("t p k -> p t k"),
                                      wt_all[:, :12, :])

            stage_b(*prev)
            nc.sync.dma_start(idx_out[12:].rearrange("t p k -> p t k"),
                              idx_u16[:, 12:, :])
            nc.sync.dma_start(wt_out[12:].rearrange("t p k -> p t k"),
                              wt_all[:, 12:, :])
    return nc


def prep_core_inputs(x_core, shared):
    n_tiles = x_core.shape[0] // 128
    x = np.ascontiguousarray(x_core, dtype=np.float32)
    xh = x.astype(np.float16)
    xl = x - xh.astype(np.float32)
    x8 = np.clip(x * np.float32(2.0 ** SX8), -240, 240).astype(NPF8)
    xl8 = np.clip(xl * np.float32(2.0 ** SXL), -240, 240).astype(NPF8)

    def tile_x(a):
        # [T, H] -> [n_tiles, 128p(h_inner), 16(h_outer), 128(t)]
        return np.ascontiguousarray(
            a.reshape(n_tiles, 128, H_CHUNKS, 128).transpose(0, 3, 2, 1))

    x88 = np.concatenate([tile_x(x8), tile_x(xl8)], axis=3)
    return {"xh": tile_x(xh), "x88": x88, **shared}


def prep_shared(weight, bias_vec):
    ws = np.ascontiguousarray(weight, dtype=np.float32) * 1024.0
    wh_ = ws.astype(np.float16)
    wl_ = ws - wh_.astype(np.float32)
    wl8 = np.clip(wl_ * np.float32(2.0 ** SWL), -240, 240).astype(NPF8)
    wh8 = np.clip(wh_.astype(np.float32) * np.float32(2.0 ** SWH),
                  -240, 240).astype(NPF8)

    def tile_w(a):
        # [E, H] -> [H, E] -> [128p(h_inner), 16(h_outer), E]
        return np.ascontiguousarray(a.T.reshape(H_CHUNKS, 128, E).transpose(1, 0, 2))

    w88 = np.concatenate([tile_w(wl8), tile_w(wh8)], axis=2)
    b = np.asarray(bias_vec, np.float32)
    bias_rep = np.broadcast_to(b, (128, E)).copy()

    # 4-bit packed bias: pbq in 0..15, quantum 2^-24 (stays below the 2^-19
    # ranking quantum so it never perturbs rank order beyond a tiebreak)
    bmin = np.float32(b.min())
    bmax = np.float32(b.max())
    step = np.float32((bmax - bmin) / 15.0) if bmax > bmin else np.float32(1.0)
    pbq = np.clip(np.round((b - bmin) / step), 0, 15).astype(np.float32)
    pb_rep = np.broadcast_to((pbq * np.float32(2.0 ** -24)).astype(np.float32),
                             (128, E)).copy()
    rec_const = np.broadcast_to(
        np.array([-np.float32(2.0 ** 24) * step, bmin], np.float32), (128, 2)).copy()
    return {"wh16": tile_w(wh_), "w88": w88, "bias_rep": bias_rep,
            "pb_rep": pb_rep, "rec_const": rec_const}


_CACHED = {}


def _get_nc():
    if "nc" not in _CACHED:
        nc = bacc.Bacc("TRN2", num_devices=N_CORES)
        build_kernel(nc)
        nc.compile()
        _CACHED["nc"] = nc
    return _CACHED["nc"]


def make_in_maps(hidden_states, weight, e_score_correction_bias):
    x = np.asarray(hidden_states, np.float32).reshape(-1, H)
    shared = prep_shared(np.asarray(weight, np.float32),
                         np.asarray(e_score_correction_bias, np.float32))
    return [prep_core_inputs(x[c * T_CORE:(c + 1) * T_CORE], shared)
            for c in range(N_CORES)]


def kernel(hidden_states, weight, e_score_correction_bias):
    in_maps = make_in_maps(hidden_states, weight, e_score_correction_bias)
    nc = _get_nc()
    res = bass_utils.run_bass_kernel_spmd(nc, in_maps, core_ids=list(range(N_CORES)))
    idx = np.concatenate([r["idx_out"].reshape(-1, 8) for r in res.results], axis=0)
    wt = np.concatenate([r["wt_out"].reshape(-1, 8) for r in res.results], axis=0)
    return idx.astype(np.int32), wt.astype(np.float32)


# revision 22
# speedup vs baseline: 1.0461x; 1.0461x over previous
"""KimiMoEGate (sigmoid scoring, group-limited top-k) on 8 Trainium2 cores.

Strategy (hardcoded for hidden_states [4,4096,2048], weight [256,2048]):
  - Token-parallel: 16384 tokens sharded 2048/core across 8 cores; router
    weight + bias replicated per core.
  - Router logits at ~fp32 accuracy in 2.0 fp16-pass-equivalents on the PE:
      main:  xh(fp16) @ wh(fp16)          -> P0   (16 matmuls, 1 cyc/row)
      corrA: x8(fp8)  @ wl8(fp8)  2^-16   -> Pc   (8 DoubleRow matmuls)
      corrB: xl8(fp8) @ wh8(fp8)  2^-16   -> Pc   (8 DoubleRow matmuls)
    where xh=fp16(x), xl=x-xh, wh=fp16(1024W), wl=1024W-wh, and the fp8
    operands carry power-of-2 scales (x8: 2^5, wl8: 2^11, xl8: 2^16,
    wh8: 2^0) so both corrections land in one PSUM at scale 2^16.
  - ACT applies sigmoid (scale 2^-10 folds away the 1024). Group-limited
    top-k via DVE max8 / max_index / match_replace on exact fp32 biased
    scores.
  - Bit-packing instead of a per-expert gather: the ranking tensor is
    rne(sfc*2^19)/2^19 + pbq[e]*2^-24 with pbq the expert bias quantized to
    4 bits. The top-8 values then carry their own bias: a +24/-24 magic
    round-trip splits them back into quantized score and packed bias, so
    w_j needs no gather.  Scale/bias-only pack steps run on the ACT engine.
  - Pipeline: tiny PE warm-up matmuls start the ramp clock; router weights
    stream in 4 chunked DMAs behind the first x tile; per-tile weight
    normalization and split output DMAs shorten the tail.
"""

import numpy as np
import ml_dtypes

from concourse import bacc, bass_utils
import concourse.mybir as mybir
from concourse.tile import TileContext

F16 = mybir.dt.float16
F32 = mybir.dt.float32
F8 = mybir.dt.float8e4
U16 = mybir.dt.uint16
AF = mybir.ActivationFunctionType
ALU = mybir.AluOpType
AX = mybir.AxisListType
NPF8 = ml_dtypes.float8_e4m3

N_CORES = 8
N_GROUP = 8
EXP_PER_GROUP = 32
E = 256
H = 2048
H_CHUNKS = 16  # 2048 / 128
T_TOTAL = 16384
T_CORE = T_TOTAL // N_CORES
N_TILES = T_CORE // 128  # 16
N_WARM = 6

MAGIC = float(1.5 * 2.0 ** 23)  # integer-rounding magic at the 2^19 scale
QOFF = float(1.5 * 2.0 ** 4)    # same magic at the v0 scale: rounds to 2^-19

SX8 = 5    # x8  = fp8(x * 2^5)
SWL = 11   # wl8 = fp8(wl * 2^11)
SXL = 16   # xl8 = fp8(xl * 2^16)
SWH = 0    # wh8 = fp8(wh)
SCORR = float(2.0 ** -(SX8 + SWL))  # = 2^-16 = 2^-(SXL+SWH)


def build_kernel(nc, n_tiles=N_TILES):
    xh = nc.dram_tensor("xh", [n_tiles, 128, H_CHUNKS, 128], F16, kind="ExternalInput").ap()
    # x88[..., 0:128] = x8 chunk (x * 2^5), x88[..., 128:256] = xl8 chunk (xl * 2^16)
    x88 = nc.dram_tensor("x88", [n_tiles, 128, H_CHUNKS, 256], F8, kind="ExternalInput").ap()
    wh16 = nc.dram_tensor("wh16", [128, H_CHUNKS, E], F16, kind="ExternalInput").ap()
    # w88[:, ho, 0:256] = wl8 chunk (wl * 2^11), w88[:, ho, 256:512] = wh8 chunk
    w88 = nc.dram_tensor("w88", [128, H_CHUNKS, 2 * E], F8, kind="ExternalInput").ap()
    bias = nc.dram_tensor("bias_rep", [128, E], F32, kind="ExternalInput").ap()
    pbt = nc.dram_tensor("pb_rep", [128, E], F32, kind="ExternalInput").ap()
    rec = nc.dram_tensor("rec_const", [128, 2], F32, kind="ExternalInput").ap()
    idx_out = nc.dram_tensor("idx_out", [n_tiles, 128, 8], U16, kind="ExternalOutput").ap()
    wt_out = nc.dram_tensor("wt_out", [n_tiles, 128, 8], F32, kind="ExternalOutput").ap()

    with TileContext(nc) as tc:
        with (
            tc.tile_pool(name="const", bufs=1) as cpool,
            tc.tile_pool(name="xin", bufs=6) as xpool,
            tc.tile_pool(name="work", bufs=6) as wpool,
            tc.tile_pool(name="psum", bufs=4, space="PSUM") as ppool,
            tc.tile_pool(name="cpsum", bufs=3, space="PSUM") as cppool,
            tc.tile_pool(name="warmps", bufs=1, space="PSUM") as wppool,
            tc.tile_pool(name="persist", bufs=1) as perspool,
        ):
            # --- PE warm-up: tiny dummy matmuls with no DMA dependency;
            # they start the PE ramp clock before the first DMAs land.
            dummy = cpool.tile([128, 64], F16)
            nc.gpsimd.memset(dummy, 0.0)
            warm_ps = wppool.tile([128, 64], F32)
            for _ in range(N_WARM):
                nc.tensor.matmul(warm_ps[0:1, :], dummy[:, 0:1], dummy,
                                 start=True, stop=True)

            wh_sb = cpool.tile([128, H_CHUNKS, E], F16)
            w88_sb = cpool.tile([128, H_CHUNKS, 2 * E], F8)
            bias_in = cpool.tile([128, E], F32)
            bias_sb = cpool.tile([128, E], F32)
            pb_in = cpool.tile([128, E], F32)
            pb_sb = cpool.tile([128, E], F32)
            rec_in = cpool.tile([128, 2], F32)
            rec_sb = cpool.tile([128, 2], F32)

            idx_u16 = perspool.tile([128, n_tiles, 8], U16)
            wt_all = perspool.tile([128, n_tiles, 8], F32)

            xtiles = {}

            def fetch(i):
                if i >= n_tiles:
                    return
                a = xpool.tile([128, H_CHUNKS, 128], F16, tag="xh")
                b = xpool.tile([128, H_CHUNKS, 256], F8, tag="x88")
                nc.sync.dma_start(a, xh[i])
                nc.sync.dma_start(b, x88[i])
                xtiles[i] = (a, b)

            # head order: xh0 + main weights first (main pass can start),
            # fp8 payloads next, consts last.
            xh0 = xpool.tile([128, H_CHUNKS, 128], F16, tag="xh")
            x880 = xpool.tile([128, H_CHUNKS, 256], F8, tag="x88")
            nc.sync.dma_start(xh0, xh[0])
            for q in range(4):
                sl = slice(4 * q, 4 * q + 4)
                nc.sync.dma_start(wh_sb[:, sl, :], wh16[:, sl, :])
            nc.sync.dma_start(x880, x88[0])
            for q in range(2):
                sl = slice(8 * q, 8 * q + 8)
                nc.sync.dma_start(w88_sb[:, sl, :], w88[:, sl, :])
            xtiles[0] = (xh0, x880)
            nc.sync.dma_start(bias_in, bias)
            nc.sync.dma_start(pb_in, pbt)
            nc.sync.dma_start(rec_in, rec)
            fetch(1)
            # engine-local copies so in-loop consumers depend on same-engine
            # producers (program order) instead of carrying DMA-sem waits.
            nc.gpsimd.tensor_copy(bias_sb, bias_in)
            nc.gpsimd.tensor_copy(pb_sb, pb_in)
            nc.vector.tensor_copy(rec_sb, rec_in)

            def stage_b(i, sfc):
                sfcg = sfc.rearrange("p (g e) -> p g e", g=N_GROUP)

                # packed ranking tensor: vq = rne(sfc*2^19)*2^-19 + pb
                # (scale/bias steps on ACT)
                mt = wpool.tile([128, E], F32, tag="mt")
                nc.scalar.activation(mt, sfc, AF.Copy, bias=MAGIC,
                                     scale=float(2.0 ** 19))
                v0 = wpool.tile([128, E], F32, tag="v0")
                nc.scalar.activation(v0, mt, AF.Copy, bias=-QOFF,
                                     scale=float(2.0 ** -19))
                vq = wpool.tile([128, E], F32, tag="vq")
                nc.gpsimd.tensor_add(vq, v0, pb_sb)

                # group stage on exact sfc (DVE): top-2 per group of 32
                g1 = wpool.tile([128, N_GROUP], F32, tag="g1")
                nc.vector.reduce_max(g1, sfcg, axis=AX.X)
                kn = wpool.tile([128, E], F32, tag="kn")
                nc.vector.match_replace(out=kn, in_to_replace=g1, in_values=sfc,
                                        imm_value=-1e30)
                g2 = wpool.tile([128, N_GROUP], F32, tag="g2")
                nc.vector.reduce_max(g2, kn.rearrange("p (g e) -> p g e", g=N_GROUP),
                                     axis=AX.X)
                gs = wpool.tile([128, N_GROUP], F32, tag="gs")
                nc.vector.tensor_add(gs, g1, g2)

                # top-4 groups: threshold at 4th largest of the 8 group scores
                g8 = wpool.tile([128, 8], F32, tag="g8")
                nc.vector.max(out=g8, in_=gs)
                gm = wpool.tile([128, N_GROUP], F32, tag="gm")
                nc.vector.tensor_scalar(gm, gs, g8[:, 3:4], None, op0=ALU.is_ge)

                # mask the packed scores and take top-8
                tmp = wpool.tile([128, N_GROUP, EXP_PER_GROUP], F32, tag="tmp")
                nc.gpsimd.tensor_mul(tmp, vq.rearrange("p (g e) -> p g e", g=N_GROUP),
                                     gm.unsqueeze(2).to_broadcast([128, N_GROUP, EXP_PER_GROUP]))
                tmpf = tmp.rearrange("p g e -> p (g e)")
                m8 = wpool.tile([128, 8], F32, tag="m8")
                nc.vector.max(out=m8, in_=tmpf)
                nc.vector.max_index(idx_u16[:, i, :], m8, tmpf)

                # unpack (ACT magic round-trip): q8 = quantized sfc
                t1 = wpool.tile([128, 8], F32, tag="t1")
                nc.vector.tensor_scalar_add(t1, m8, QOFF)
                q8 = wpool.tile([128, 8], F32, tag="q8")
                nc.vector.tensor_scalar_sub(q8, t1, QOFF)
                pbv = wpool.tile([128, 8], F32, tag="pbv")
                nc.vector.tensor_sub(pbv, m8, q8)
                # w = q8 - (pbv * 2^24 * step + bmin)
                #   = (pbv * rec0 + q8) - rec1  [rec0 = -2^24*step, rec1 = bmin]
                u2 = wpool.tile([128, 8], F32, tag="u2")
                nc.vector.scalar_tensor_tensor(out=u2, in0=pbv,
                                               scalar=rec_sb[:, 0:1], in1=q8,
                                               op0=ALU.mult, op1=ALU.add)
                wr = wpool.tile([128, 8], F32, tag="wr")
                nc.vector.tensor_scalar(wr, u2, rec_sb[:, 1:2], None,
                                        op0=ALU.subtract)
                s = wpool.tile([128, 1], F32, tag="s")
                nc.vector.reduce_sum(s, wr.rearrange("p (o k) -> p o k", o=1),
                                     axis=AX.X)
                r = wpool.tile([128, 1], F32, tag="r")
                nc.vector.reciprocal(r, s)
                nc.vector.tensor_scalar(wt_all[:, i, :], wr, r[:, 0:1], 2.5,
                                        op0=ALU.mult, op1=ALU.mult)

            prev = None
            for i in range(n_tiles):
                fetch(i + 2)
                xh_sb, x88_sb = xtiles.pop(i)

                # main pass: P0 = xh @ wh  (1024 * logit, fp16 operands)
                p0 = ppool.tile([128, E], F32)
                for ho in range(H_CHUNKS):
                    nc.tensor.matmul(p0, xh_sb[:, ho, :], wh_sb[:, ho, :],
                                     start=(ho == 0), stop=(ho == H_CHUNKS - 1))
                # corrections: Pc = x8 @ wl8 + xl8 @ wh8 (DoubleRow fp8,
                # both at scale 2^16 relative to P0)
                pc = cppool.tile([128, E], F32)
                for hp in range(H_CHUNKS // 2):
                    sl = slice(2 * hp, 2 * hp + 2)
                    nc.tensor.matmul(pc, x88_sb[:, sl, 0:128], w88_sb[:, sl, 0:E],
                                     start=(hp == 0), stop=False,
                                     perf_mode=mybir.MatmulPerfMode.DoubleRow)
                for hp in range(H_CHUNKS // 2):
                    sl = slice(2 * hp, 2 * hp + 2)
                    nc.tensor.matmul(pc, x88_sb[:, sl, 128:256], w88_sb[:, sl, E:],
                                     start=False, stop=(hp == H_CHUNKS // 2 - 1),
                                     perf_mode=mybir.MatmulPerfMode.DoubleRow)

                # stage A: u = P0 + Pc*2^-16; scores = sigmoid(u*2^-10);
                # sfc = scores + bias.  Emitted ahead of the previous tile's
                # post-chain so the ACT->DVE->ACT ring never stalls the
                # next tile's sigmoid.
                u1 = wpool.tile([128, E], F32, tag="u1")
                nc.scalar.activation(u1, pc, AF.Copy, scale=SCORR)
                u = wpool.tile([128, E], F32, tag="u")
                nc.vector.tensor_add(u, u1, p0)
                scores = wpool.tile([128, E], F32, tag="scores")
                nc.scalar.activation(scores, u, AF.Sigmoid, scale=float(2.0 ** -10))
                sfc = wpool.tile([128, E], F32, tag="sfc")
                nc.gpsimd.tensor_add(sfc, scores, bias_sb)

                if prev is not None:
                    stage_b(*prev)
                prev = (i, sfc)

                if i == 13:
                    # input prefetch is done (fetch(15) just issued); SP is
                    # free from here, so these waits block nothing.
                    nc.sync.dma_start(idx_out[:12].rearrange("t p k -> p t k"),
                                      idx_u16[:, :12, :])
                    nc.sync.dma_start(wt_out[:12].rearrange("t p k -> p t k"),
                                      wt_all[:, :12, :])

            stage_b(*prev)
            nc.sync.dma_start(idx_out[12:].rearrange("t p k -> p t k"),
                              idx_u16[:, 12:, :])
            nc.sync.dma_start(wt_out[12:].rearrange("t p k -> p t k"),
                              wt_all[:, 12:, :])
    return nc


def prep_core_inputs(x_core, shared):
    n_tiles = x_core.shape[0] // 128
    x = np.ascontiguousarray(x_core, dtype=np.float32)
    xh = x.astype(np.float16)
    xl = x - xh.astype(np.float32)
    x8 = np.clip(x * np.float32(2.0 ** SX8), -240, 240).astype(NPF8)
    xl8 = np.clip(xl * np.float32(2.0 ** SXL), -240, 240).astype(NPF8)

    def tile_x(a):
        # [T, H] -> [n_tiles, 128p(h_inner), 16(h_outer), 128(t)]
        return np.ascontiguousarray(
            a.reshape(n_tiles, 128, H_CHUNKS, 128).transpose(0, 3, 2, 1))

    x88 = np.concatenate([tile_x(x8), tile_x(xl8)], axis=3)
    return {"xh": tile_x(xh), "x88": x88, **shared}


def prep_shared(weight, bias_vec):
    ws = np.ascontiguousarray(weight, dtype=np.float32) * 1024.0
    wh_ = ws.astype(np.float16)
    wl_ = ws - wh_.astype(np.float32)
    wl8 = np.clip(wl_ * np.float32(2.0 ** SWL), -240, 240).astype(NPF8)
    wh8 = np.clip(wh_.astype(np.float32) * np.float32(2.0 ** SWH),
                  -240, 240).astype(NPF8)

    def tile_w(a):
        # [E, H] -> [H, E] -> [128p(h_inner), 16(h_outer), E]
        return np.ascontiguousarray(a.T.reshape(H_CHUNKS, 128, E).transpose(1, 0, 2))

    w88 = np.concatenate([tile_w(wl8), tile_w(wh8)], axis=2)
    b = np.asarray(bias_vec, np.float32)
    bias_rep = np.broadcast_to(b, (128, E)).copy()

    # 4-bit packed bias: pbq in 0..15, quantum 2^-24 (stays below the 2^-19
    # ranking quantum so it never perturbs rank order beyond a tiebreak)
    bmin = np.float32(b.min())
    bmax = np.float32(b.max())
    step = np.float32((bmax - bmin) / 15.0) if bmax > bmin else np.float32(1.0)
    pbq = np.clip(np.round((b - bmin) / step), 0, 15).astype(np.float32)
    pb_rep = np.broadcast_to((pbq * np.float32(2.0 ** -24)).astype(np.float32),
                             (128, E)).copy()
    rec_const = np.broadcast_to(
        np.array([-np.float32(2.0 ** 24) * step, bmin], np.float32), (128, 2)).copy()
    return {"wh16": tile_w(wh_), "w88": w88, "bias_rep": bias_rep,
            "pb_rep": pb_rep, "rec_const": rec_const}


_CACHED = {}


def _get_nc():
    if "nc" not in _CACHED:
        nc = bacc.Bacc("TRN2", num_devices=N_CORES)
        build_kernel(nc)
        nc.compile()
        _CACHED["nc"] = nc
    return _CACHED["nc"]


def make_in_maps(hidden_states, weight, e_score_correction_bias):
    x = np.asarray(hidden_states, np.float32).reshape(-1, H)
    shared = prep_shared(np.asarray(weight, np.float32),
                         np.asarray(e_score_correction_bias, np.float32))
    return [prep_core_inputs(x[c * T_CORE:(c + 1) * T_CORE], shared)
            for c in range(N_CORES)]


def kernel(hidden_states, weight, e_score_correction_bias):
    in_maps = make_in_maps(hidden_states, weight, e_score_correction_bias)
    nc = _get_nc()
    res = bass_utils.run_bass_kernel_spmd(nc, in_maps, core_ids=list(range(N_CORES)))
    idx = np.concatenate([r["idx_out"].reshape(-1, 8) for r in res.results], axis=0)
    wt = np.concatenate([r["wt_out"].reshape(-1, 8) for r in res.results], axis=0)
    return idx.astype(np.int32), wt.astype(np.float32)


# revision 25
# speedup vs baseline: 1.0525x; 1.0061x over previous
"""KimiMoEGate (sigmoid scoring, group-limited top-k) on 8 Trainium2 cores.

Strategy (hardcoded for hidden_states [4,4096,2048], weight [256,2048]):
  - Token-parallel: 16384 tokens sharded 2048/core across 8 cores; router
    weight + bias replicated per core.
  - Router logits at ~fp32 accuracy in 2.0 fp16-pass-equivalents on the PE:
      main:  xh(fp16) @ wh(fp16)          -> P0   (16 matmuls, 1 cyc/row)
      corrA: x8(fp8)  @ wl8(fp8)  2^-16   -> Pc   (8 DoubleRow matmuls)
      corrB: xl8(fp8) @ wh8(fp8)  2^-16   -> Pc   (8 DoubleRow matmuls)
    where xh=fp16(x), xl=x-xh, wh=fp16(1024W), wl=1024W-wh, and the fp8
    operands carry power-of-2 scales (x8: 2^5, wl8: 2^11, xl8: 2^16,
    wh8: 2^0) so both corrections land in one PSUM at scale 2^16.
  - ACT applies sigmoid (scale 2^-10 folds away the 1024). Group-limited
    top-k via DVE max8 / max_index / match_replace on exact fp32 biased
    scores.
  - Bit-packing instead of a per-expert gather: the ranking tensor is
    rne(sfc*2^19)/2^19 + pbq[e]*2^-24 with pbq the expert bias quantized to
    4 bits. The top-8 values then carry their own bias: a +24/-24 magic
    round-trip splits them back into quantized score and packed bias, so
    w_j needs no gather.  Scale/bias-only pack steps run on the ACT engine.
  - Pipeline: tiny PE warm-up matmuls start the ramp clock; router weights
    stream in 4 chunked DMAs behind the first x tile; per-tile weight
    normalization and split output DMAs shorten the tail.
"""

import numpy as np
import ml_dtypes

from concourse import bacc, bass_utils
import concourse.mybir as mybir
from concourse.tile import TileContext

F16 = mybir.dt.float16
F32 = mybir.dt.float32
F8 = mybir.dt.float8e4
U16 = mybir.dt.uint16
AF = mybir.ActivationFunctionType
ALU = mybir.AluOpType
AX = mybir.AxisListType
NPF8 = ml_dtypes.float8_e4m3

N_CORES = 8
N_GROUP = 8
EXP_PER_GROUP = 32
E = 256
H = 2048
H_CHUNKS = 16  # 2048 / 128
T_TOTAL = 16384
T_CORE = T_TOTAL // N_CORES
N_TILES = T_CORE // 128  # 16
N_WARM = 6

MAGIC = float(1.5 * 2.0 ** 23)  # integer-rounding magic at the 2^19 scale
QOFF = float(1.5 * 2.0 ** 4)    # same magic at the v0 scale: rounds to 2^-19

SX8 = 5    # x8  = fp8(x * 2^5)
SWL = 11   # wl8 = fp8(wl * 2^11)
SXL = 16   # xl8 = fp8(xl * 2^16)
SWH = 0    # wh8 = fp8(wh)
SCORR = float(2.0 ** -(SX8 + SWL))  # = 2^-16 = 2^-(SXL+SWH)


def build_kernel(nc, n_tiles=N_TILES):
    xh = nc.dram_tensor("xh", [n_tiles, 128, H_CHUNKS, 128], F16, kind="ExternalInput").ap()
    # x88[..., 0:128] = x8 chunk (x * 2^5), x88[..., 128:256] = xl8 chunk (xl * 2^16)
    x88 = nc.dram_tensor("x88", [n_tiles, 128, H_CHUNKS, 256], F8, kind="ExternalInput").ap()
    wh16 = nc.dram_tensor("wh16", [128, H_CHUNKS, E], F16, kind="ExternalInput").ap()
    # w88[:, ho, 0:256] = wl8 chunk (wl * 2^11), w88[:, ho, 256:512] = wh8 chunk
    w88 = nc.dram_tensor("w88", [128, H_CHUNKS, 2 * E], F8, kind="ExternalInput").ap()
    bias = nc.dram_tensor("bias_rep", [128, E], F32, kind="ExternalInput").ap()
    pbt = nc.dram_tensor("pb_rep", [128, E], F32, kind="ExternalInput").ap()
    rec = nc.dram_tensor("rec_const", [128, 2], F32, kind="ExternalInput").ap()
    idx_out = nc.dram_tensor("idx_out", [n_tiles, 128, 8], U16, kind="ExternalOutput").ap()
    wt_out = nc.dram_tensor("wt_out", [n_tiles, 128, 8], F32, kind="ExternalOutput").ap()

    with TileContext(nc) as tc:
        with (
            tc.tile_pool(name="const", bufs=1) as cpool,
            tc.tile_pool(name="xin", bufs=6) as xpool,
            tc.tile_pool(name="work", bufs=6) as wpool,
            tc.tile_pool(name="psum", bufs=5, space="PSUM") as ppool,
            tc.tile_pool(name="cpsum", bufs=2, space="PSUM") as cppool,
            tc.tile_pool(name="warmps", bufs=1, space="PSUM") as wppool,
            tc.tile_pool(name="persist", bufs=1) as perspool,
        ):
            # --- PE warm-up: tiny dummy matmuls with no DMA dependency;
            # they start the PE ramp clock before the first DMAs land.
            dummy = cpool.tile([128, 64], F16)
            nc.gpsimd.memset(dummy, 0.0)
            warm_ps = wppool.tile([128, 64], F32)
            for _ in range(N_WARM):
                nc.tensor.matmul(warm_ps[0:1, :], dummy[:, 0:1], dummy,
                                 start=True, stop=True)

            wh_sb = cpool.tile([128, H_CHUNKS, E], F16)
            w88_sb = cpool.tile([128, H_CHUNKS, 2 * E], F8)
            bias_in = cpool.tile([128, E], F32)
            bias_sb = cpool.tile([128, E], F32)
            pb_in = cpool.tile([128, E], F32)
            pb_sb = cpool.tile([128, E], F32)
            rec_in = cpool.tile([128, 2], F32)
            rec_sb = cpool.tile([128, 2], F32)

            idx_u16 = perspool.tile([128, n_tiles, 8], U16)
            wt_all = perspool.tile([128, n_tiles, 8], F32)

            xtiles = {}

            def fetch(i):
                if i >= n_tiles:
                    return
                a = xpool.tile([128, H_CHUNKS, 128], F16, tag="xh")
                b = xpool.tile([128, H_CHUNKS, 256], F8, tag="x88")
                nc.sync.dma_start(a, xh[i])
                nc.sync.dma_start(b, x88[i])
                xtiles[i] = (a, b)

            # head order: xh0 + main weights first (main pass can start),
            # fp8 payloads next, consts last.
            xh0 = xpool.tile([128, H_CHUNKS, 128], F16, tag="xh")
            x880 = xpool.tile([128, H_CHUNKS, 256], F8, tag="x88")
            nc.sync.dma_start(xh0, xh[0])
            for q in range(4):
                sl = slice(4 * q, 4 * q + 4)
                nc.sync.dma_start(wh_sb[:, sl, :], wh16[:, sl, :])
            nc.sync.dma_start(x880, x88[0])
            for q in range(2):
                sl = slice(8 * q, 8 * q + 8)
                nc.sync.dma_start(w88_sb[:, sl, :], w88[:, sl, :])
            xtiles[0] = (xh0, x880)
            nc.sync.dma_start(bias_in, bias)
            nc.sync.dma_start(pb_in, pbt)
            nc.sync.dma_start(rec_in, rec)
            fetch(1)
            # engine-local copies so in-loop consumers depend on same-engine
            # producers (program order) instead of carrying DMA-sem waits.
            nc.gpsimd.tensor_copy(bias_sb, bias_in)
            nc.gpsimd.tensor_copy(pb_sb, pb_in)
            nc.vector.tensor_copy(rec_sb, rec_in)

            def stage_b(i, sfc):
                sfcg = sfc.rearrange("p (g e) -> p g e", g=N_GROUP)

                # packed ranking tensor: vq = rne(sfc*2^19)*2^-19 + pb
                # (scale/bias steps on ACT)
                mt = wpool.tile([128, E], F32, tag="mt")
                nc.scalar.activation(mt, sfc, AF.Copy, bias=MAGIC,
                                     scale=float(2.0 ** 19))
                v0 = wpool.tile([128, E], F32, tag="v0")
                nc.scalar.activation(v0, mt, AF.Copy, bias=-QOFF,
                                     scale=float(2.0 ** -19))
                vq = wpool.tile([128, E], F32, tag="vq")
                nc.gpsimd.tensor_add(vq, v0, pb_sb)

                # group stage on exact sfc (DVE): top-2 per group of 32
                g1 = wpool.tile([128, N_GROUP], F32, tag="g1")
                nc.vector.reduce_max(g1, sfcg, axis=AX.X)
                kn = wpool.tile([128, E], F32, tag="kn")
                nc.vector.match_replace(out=kn, in_to_replace=g1, in_values=sfc,
                                        imm_value=-1e30)
                g2 = wpool.tile([128, N_GROUP], F32, tag="g2")
                nc.vector.reduce_max(g2, kn.rearrange("p (g e) -> p g e", g=N_GROUP),
                                     axis=AX.X)
                gs = wpool.tile([128, N_GROUP], F32, tag="gs")
                nc.vector.tensor_add(gs, g1, g2)

                # top-4 groups: threshold at 4th largest of the 8 group scores
                g8 = wpool.tile([128, 8], F32, tag="g8")
                nc.vector.max(out=g8, in_=gs)
                gm = wpool.tile([128, N_GROUP], F32, tag="gm")
                nc.vector.tensor_scalar(gm, gs, g8[:, 3:4], None, op0=ALU.is_ge)

                # mask the packed scores and take top-8
                tmp = wpool.tile([128, N_GROUP, EXP_PER_GROUP], F32, tag="tmp")
                nc.gpsimd.tensor_mul(tmp, vq.rearrange("p (g e) -> p g e", g=N_GROUP),
                                     gm.unsqueeze(2).to_broadcast([128, N_GROUP, EXP_PER_GROUP]))
                tmpf = tmp.rearrange("p g e -> p (g e)")
                m8 = wpool.tile([128, 8], F32, tag="m8")
                nc.vector.max(out=m8, in_=tmpf)
                nc.vector.max_index(idx_u16[:, i, :], m8, tmpf)

                # unpack (ACT magic round-trip): q8 = quantized sfc
                t1 = wpool.tile([128, 8], F32, tag="t1")
                nc.scalar.activation(t1, m8, AF.Copy, bias=QOFF)
                q8 = wpool.tile([128, 8], F32, tag="q8")
                nc.scalar.activation(q8, t1, AF.Copy, bias=-QOFF)
                pbv = wpool.tile([128, 8], F32, tag="pbv")
                nc.vector.tensor_sub(pbv, m8, q8)
                # w = q8 - (pbv * 2^24 * step + bmin)
                #   = (pbv * rec0 + q8) - rec1  [rec0 = -2^24*step, rec1 = bmin]
                u2 = wpool.tile([128, 8], F32, tag="u2")
                nc.vector.scalar_tensor_tensor(out=u2, in0=pbv,
                                               scalar=rec_sb[:, 0:1], in1=q8,
                                               op0=ALU.mult, op1=ALU.add)
                wr = wpool.tile([128, 8], F32, tag="wr")
                nc.vector.tensor_scalar(wr, u2, rec_sb[:, 1:2], None,
                                        op0=ALU.subtract)
                s = wpool.tile([128, 1], F32, tag="s")
                nc.vector.reduce_sum(s, wr.rearrange("p (o k) -> p o k", o=1),
                                     axis=AX.X)
                r = wpool.tile([128, 1], F32, tag="r")
                nc.vector.reciprocal(r, s)
                nc.vector.tensor_scalar(wt_all[:, i, :], wr, r[:, 0:1], 2.5,
                                        op0=ALU.mult, op1=ALU.mult)

            prev = None
            for i in range(n_tiles):
                fetch(i + 2)
                xh_sb, x88_sb = xtiles.pop(i)

                # main pass: P0 = xh @ wh  (1024 * logit, fp16 operands)
                p0 = ppool.tile([128, E], F32)
                for ho in range(H_CHUNKS):
                    nc.tensor.matmul(p0, xh_sb[:, ho, :], wh_sb[:, ho, :],
                                     start=(ho == 0), stop=(ho == H_CHUNKS - 1))
                # corrections: Pc = x8 @ wl8 + xl8 @ wh8 (DoubleRow fp8,
                # both at scale 2^16 relative to P0)
                pc = cppool.tile([128, E], F32)
                for hp in range(H_CHUNKS // 2):
                    sl = slice(2 * hp, 2 * hp + 2)
                    nc.tensor.matmul(pc, x88_sb[:, sl, 0:128], w88_sb[:, sl, 0:E],
                                     start=(hp == 0), stop=False,
                                     perf_mode=mybir.MatmulPerfMode.DoubleRow)
                for hp in range(H_CHUNKS // 2):
                    sl = slice(2 * hp, 2 * hp + 2)
                    nc.tensor.matmul(pc, x88_sb[:, sl, 128:256], w88_sb[:, sl, E:],
                                     start=False, stop=(hp == H_CHUNKS // 2 - 1),
                                     perf_mode=mybir.MatmulPerfMode.DoubleRow)

                # stage A: u = P0 + Pc*2^-16; scores = sigmoid(u*2^-10);
                # sfc = scores + bias.  Emitted ahead of the previous tile's
                # post-chain so the ACT->DVE->ACT ring never stalls the
                # next tile's sigmoid.
                u1 = wpool.tile([128, E], F32, tag="u1")
                nc.scalar.activation(u1, pc, AF.Copy, scale=SCORR)
                u = wpool.tile([128, E], F32, tag="u")
                nc.vector.tensor_add(u, u1, p0)
                scores = wpool.tile([128, E], F32, tag="scores")
                nc.scalar.activation(scores, u, AF.Sigmoid, scale=float(2.0 ** -10))
                sfc = wpool.tile([128, E], F32, tag="sfc")
                nc.gpsimd.tensor_add(sfc, scores, bias_sb)

                if prev is not None:
                    stage_b(*prev)
                prev = (i, sfc)

                if i == 13:
                    # input prefetch is done (fetch(15) just issued); SP is
                    # free from here, so these waits block nothing.
                    nc.sync.dma_start(idx_out[:12].rearrange("t p k -> p t k"),
                                      idx_u16[:, :12, :])
                    nc.sync.dma_start(wt_out[:12].rearrange("t p k -> p t k"),
                                      wt_all[:, :12, :])

            stage_b(*prev)
            nc.sync.dma_start(idx_out[12:].rearrange("t p k -> p t k"),
                              idx_u16[:, 12:, :])
            nc.sync.dma_start(wt_out[12:].rearrange("t p k -> p t k"),
                              wt_all[:, 12:, :])
    return nc


def prep_core_inputs(x_core, shared):
    n_tiles = x_core.shape[0] // 128
    x = np.ascontiguousarray(x_core, dtype=np.float32)
    xh = x.astype(np.float16)
    xl = x - xh.astype(np.float32)
    x8 = np.clip(x * np.float32(2.0 ** SX8), -240, 240).astype(NPF8)
    xl8 = np.clip(xl * np.float32(2.0 ** SXL), -240, 240).astype(NPF8)

    def tile_x(a):
        # [T, H] -> [n_tiles, 128p(h_inner), 16(h_outer), 128(t)]
        return np.ascontiguousarray(
            a.reshape(n_tiles, 128, H_CHUNKS, 128).transpose(0, 3, 2, 1))

    x88 = np.concatenate([tile_x(x8), tile_x(xl8)], axis=3)
    return {"xh": tile_x(xh), "x88": x88, **shared}


def prep_shared(weight, bias_vec):
    ws = np.ascontiguousarray(weight, dtype=np.float32) * 1024.0
    wh_ = ws.astype(np.float16)
    wl_ = ws - wh_.astype(np.float32)
    wl8 = np.clip(wl_ * np.float32(2.0 ** SWL), -240, 240).astype(NPF8)
    wh8 = np.clip(wh_.astype(np.float32) * np.float32(2.0 ** SWH),
                  -240, 240).astype(NPF8)

    def tile_w(a):
        # [E, H] -> [H, E] -> [128p(h_inner), 16(h_outer), E]
        return np.ascontiguousarray(a.T.reshape(H_CHUNKS, 128, E).transpose(1, 0, 2))

    w88 = np.concatenate([tile_w(wl8), tile_w(wh8)], axis=2)
    b = np.asarray(bias_vec, np.float32)
    bias_rep = np.broadcast_to(b, (128, E)).copy()

    # 4-bit packed bias: pbq in 0..15, quantum 2^-24 (stays below the 2^-19
    # ranking quantum so it never perturbs rank order beyond a tiebreak)
    bmin = np.float32(b.min())
    bmax = np.float32(b.max())
    step = np.float32((bmax - bmin) / 15.0) if bmax > bmin else np.float32(1.0)
    pbq = np.clip(np.round((b - bmin) / step), 0, 15).astype(np.float32)
    pb_rep = np.broadcast_to((pbq * np.float32(2.0 ** -24)).astype(np.float32),
                             (128, E)).copy()
    rec_const = np.broadcast_to(
        np.array([-np.float32(2.0 ** 24) * step, bmin], np.float32), (128, 2)).copy()
    return {"wh16": tile_w(wh_), "w88": w88, "bias_rep": bias_rep,
            "pb_rep": pb_rep, "rec_const": rec_const}


_CACHED = {}


def _get_nc():
    if "nc" not in _CACHED:
        nc = bacc.Bacc("TRN2", num_devices=N_CORES)
        build_kernel(nc)
        nc.compile()
        _CACHED["nc"] = nc
    return _CACHED["nc"]


def make_in_maps(hidden_states, weight, e_score_correction_bias):
    x = np.asarray(hidden_states, np.float32).reshape(-1, H)
    shared = prep_shared(np.asarray(weight, np.float32),
                         np.asarray(e_score_correction_bias, np.float32))
    return [prep_core_inputs(x[c * T_CORE:(c + 1) * T_CORE], shared)
            for c in range(N_CORES)]


def kernel(hidden_states, weight, e_score_correction_bias):
    in_maps = make_in_maps(hidden_states, weight, e_score_correction_bias)
    nc = _get_nc()
    res = bass_utils.run_bass_kernel_spmd(nc, in_maps, core_ids=list(range(N_CORES)))
    idx = np.concatenate([r["idx_out"].reshape(-1, 8) for r in res.results], axis=0)
    wt = np.concatenate([r["wt_out"].reshape(-1, 8) for r in res.results], axis=0)
    return idx.astype(np.int32), wt.astype(np.float32)


# revision 27
# speedup vs baseline: 1.0532x; 1.0006x over previous
"""KimiMoEGate (sigmoid scoring, group-limited top-k) on 8 Trainium2 cores.

Strategy (hardcoded for hidden_states [4,4096,2048], weight [256,2048]):
  - Token-parallel: 16384 tokens sharded 2048/core across 8 cores; router
    weight + bias replicated per core.
  - Router logits at ~fp32 accuracy in 2.0 fp16-pass-equivalents on the PE:
      main:  xh(fp16) @ wh(fp16)          -> P0   (16 matmuls, 1 cyc/row)
      corrA: x8(fp8)  @ wl8(fp8)  2^-16   -> Pc   (8 DoubleRow matmuls)
      corrB: xl8(fp8) @ wh8(fp8)  2^-16   -> Pc   (8 DoubleRow matmuls)
    where xh=fp16(x), xl=x-xh, wh=fp16(1024W), wl=1024W-wh, and the fp8
    operands carry power-of-2 scales (x8: 2^5, wl8: 2^11, xl8: 2^16,
    wh8: 2^0) so both corrections land in one PSUM at scale 2^16.
  - ACT applies sigmoid (scale 2^-10 folds away the 1024). Group-limited
    top-k via DVE max8 / max_index / match_replace on exact fp32 biased
    scores.
  - Bit-packing instead of a per-expert gather: the ranking tensor is
    rne(sfc*2^19)/2^19 + pbq[e]*2^-24 with pbq the expert bias quantized to
    4 bits. The top-8 values then carry their own bias: a +24/-24 magic
    round-trip splits them back into quantized score and packed bias, so
    w_j needs no gather.  Scale/bias-only pack steps run on the ACT engine.
  - Pipeline: tiny PE warm-up matmuls start the ramp clock; router weights
    stream in 4 chunked DMAs behind the first x tile; per-tile weight
    normalization and split output DMAs shorten the tail.
"""

import numpy as np
import ml_dtypes

from concourse import bacc, bass_utils
import concourse.mybir as mybir
from concourse.tile import TileContext

F16 = mybir.dt.float16
F32 = mybir.dt.float32
F8 = mybir.dt.float8e4
U16 = mybir.dt.uint16
AF = mybir.ActivationFunctionType
ALU = mybir.AluOpType
AX = mybir.AxisListType
NPF8 = ml_dtypes.float8_e4m3

N_CORES = 8
N_GROUP = 8
EXP_PER_GROUP = 32
E = 256
H = 2048
H_CHUNKS = 16  # 2048 / 128
T_TOTAL = 16384
T_CORE = T_TOTAL // N_CORES
N_TILES = T_CORE // 128  # 16
N_WARM = 6

MAGIC = float(1.5 * 2.0 ** 23)  # integer-rounding magic at the 2^19 scale
QOFF = float(1.5 * 2.0 ** 4)    # same magic at the v0 scale: rounds to 2^-19

SX8 = 5    # x8  = fp8(x * 2^5)
SWL = 11   # wl8 = fp8(wl * 2^11)
SXL = 16   # xl8 = fp8(xl * 2^16)
SWH = 0    # wh8 = fp8(wh)
SCORR = float(2.0 ** -(SX8 + SWL))  # = 2^-16 = 2^-(SXL+SWH)


def build_kernel(nc, n_tiles=N_TILES):
    xh = nc.dram_tensor("xh", [n_tiles, 128, H_CHUNKS, 128], F16, kind="ExternalInput").ap()
    # x88[..., 0:128] = x8 chunk (x * 2^5), x88[..., 128:256] = xl8 chunk (xl * 2^16)
    x88 = nc.dram_tensor("x88", [n_tiles, 128, H_CHUNKS, 256], F8, kind="ExternalInput").ap()
    wh16 = nc.dram_tensor("wh16", [128, H_CHUNKS, E], F16, kind="ExternalInput").ap()
    # w88[:, ho, 0:256] = wl8 chunk (wl * 2^11), w88[:, ho, 256:512] = wh8 chunk
    w88 = nc.dram_tensor("w88", [128, H_CHUNKS, 2 * E], F8, kind="ExternalInput").ap()
    bias = nc.dram_tensor("bias_rep", [128, E], F32, kind="ExternalInput").ap()
    pbt = nc.dram_tensor("pb_rep", [128, E], F32, kind="ExternalInput").ap()
    rec = nc.dram_tensor("rec_const", [128, 2], F32, kind="ExternalInput").ap()
    ident = nc.dram_tensor("ident", [128, 128], F16, kind="ExternalInput").ap()
    idx_out = nc.dram_tensor("idx_out", [n_tiles, 128, 8], U16, kind="ExternalOutput").ap()
    wt_out = nc.dram_tensor("wt_out", [n_tiles, 128, 8], F32, kind="ExternalOutput").ap()

    with TileContext(nc) as tc:
        with (
            tc.tile_pool(name="const", bufs=1) as cpool,
            tc.tile_pool(name="xin", bufs=6) as xpool,
            tc.tile_pool(name="work", bufs=6) as wpool,
            tc.tile_pool(name="psum", bufs=5, space="PSUM") as ppool,
            tc.tile_pool(name="cpsum", bufs=2, space="PSUM") as cppool,
            tc.tile_pool(name="warmps", bufs=1, space="PSUM") as wppool,
            tc.tile_pool(name="persist", bufs=1) as perspool,
        ):
            # --- PE warm-up: tiny dummy matmuls with no DMA dependency;
            # they start the PE ramp clock before the first DMAs land.
            dummy = cpool.tile([128, 64], F16)
            nc.gpsimd.memset(dummy, 0.0)
            warm_ps = wppool.tile([128, 64], F32)
            for _ in range(N_WARM):
                nc.tensor.matmul(warm_ps[0:1, :], dummy[:, 0:1], dummy,
                                 start=True, stop=True)

            wh_sb = cpool.tile([128, H_CHUNKS, E], F16)
            w88_sb = cpool.tile([128, H_CHUNKS, 2 * E], F8)
            bias_in = cpool.tile([128, E], F32)
            bias_sb = cpool.tile([128, E], F32)
            pb_in = cpool.tile([128, E], F32)
            pb_sb = cpool.tile([128, E], F32)
            rec_in = cpool.tile([128, 2], F32)
            rec_sb = cpool.tile([128, 2], F32)
            ident_sb = cpool.tile([128, 128], F16)

            idx_u16 = perspool.tile([128, n_tiles, 8], U16)
            wt_all = perspool.tile([128, n_tiles, 8], F32)

            xtiles = {}

            def fetch(i):
                if i >= n_tiles:
                    return
                a = xpool.tile([128, H_CHUNKS, 128], F16, tag="xh")
                b = xpool.tile([128, H_CHUNKS, 256], F8, tag="x88")
                nc.sync.dma_start(a, xh[i])
                nc.sync.dma_start(b, x88[i])
                xtiles[i] = (a, b)

            # head order: xh0 + main weights first (main pass can start),
            # fp8 payloads next, consts last.
            xh0 = xpool.tile([128, H_CHUNKS, 128], F16, tag="xh")
            x880 = xpool.tile([128, H_CHUNKS, 256], F8, tag="x88")
            nc.sync.dma_start(xh0, xh[0])
            for q in range(4):
                sl = slice(4 * q, 4 * q + 4)
                nc.sync.dma_start(wh_sb[:, sl, :], wh16[:, sl, :])
            nc.sync.dma_start(x880, x88[0])
            for q in range(2):
                sl = slice(8 * q, 8 * q + 8)
                nc.sync.dma_start(w88_sb[:, sl, :], w88[:, sl, :])
            xtiles[0] = (xh0, x880)
            nc.sync.dma_start(bias_in, bias)
            nc.sync.dma_start(pb_in, pbt)
            nc.sync.dma_start(rec_in, rec)
            nc.sync.dma_start(ident_sb, ident)
            fetch(1)
            # engine-local copies so in-loop consumers depend on same-engine
            # producers (program order) instead of carrying DMA-sem waits.
            nc.gpsimd.tensor_copy(bias_sb, bias_in)
            nc.gpsimd.tensor_copy(pb_sb, pb_in)
            nc.vector.tensor_copy(rec_sb, rec_in)

            def stage_b(i, sfc):
                sfcg = sfc.rearrange("p (g e) -> p g e", g=N_GROUP)

                # packed ranking tensor: vq = rne(sfc*2^19)*2^-19 + pb
                # (scale/bias steps on ACT)
                mt = wpool.tile([128, E], F32, tag="mt")
                nc.scalar.activation(mt, sfc, AF.Copy, bias=MAGIC,
                                     scale=float(2.0 ** 19))
                v0 = wpool.tile([128, E], F32, tag="v0")
                nc.scalar.activation(v0, mt, AF.Copy, bias=-QOFF,
                                     scale=float(2.0 ** -19))
                vq = wpool.tile([128, E], F32, tag="vq")
                nc.gpsimd.tensor_add(vq, v0, pb_sb)

                # group stage on exact sfc (DVE): top-2 per group of 32
                g1 = wpool.tile([128, N_GROUP], F32, tag="g1")
                nc.vector.reduce_max(g1, sfcg, axis=AX.X)
                kn = wpool.tile([128, E], F32, tag="kn")
                nc.vector.match_replace(out=kn, in_to_replace=g1, in_values=sfc,
                                        imm_value=-1e30)
                g2 = wpool.tile([128, N_GROUP], F32, tag="g2")
                nc.vector.reduce_max(g2, kn.rearrange("p (g e) -> p g e", g=N_GROUP),
                                     axis=AX.X)
                gs = wpool.tile([128, N_GROUP], F32, tag="gs")
                nc.vector.tensor_add(gs, g1, g2)

                # top-4 groups: threshold at 4th largest of the 8 group scores
                g8 = wpool.tile([128, 8], F32, tag="g8")
                nc.vector.max(out=g8, in_=gs)
                gm = wpool.tile([128, N_GROUP], F32, tag="gm")
                nc.vector.tensor_scalar(gm, gs, g8[:, 3:4], None, op0=ALU.is_ge)

                # mask the packed scores and take top-8
                tmp = wpool.tile([128, N_GROUP, EXP_PER_GROUP], F32, tag="tmp")
                nc.gpsimd.tensor_mul(tmp, vq.rearrange("p (g e) -> p g e", g=N_GROUP),
                                     gm.unsqueeze(2).to_broadcast([128, N_GROUP, EXP_PER_GROUP]))
                tmpf = tmp.rearrange("p g e -> p (g e)")
                m8 = wpool.tile([128, 8], F32, tag="m8")
                nc.vector.max(out=m8, in_=tmpf)
                nc.vector.max_index(idx_u16[:, i, :], m8, tmpf)

                # unpack (ACT magic round-trip): q8 = quantized sfc
                t1 = wpool.tile([128, 8], F32, tag="t1")
                nc.scalar.activation(t1, m8, AF.Copy, bias=QOFF)
                q8 = wpool.tile([128, 8], F32, tag="q8")
                nc.scalar.activation(q8, t1, AF.Copy, bias=-QOFF)
                pbv = wpool.tile([128, 8], F32, tag="pbv")
                nc.vector.tensor_sub(pbv, m8, q8)
                # w = q8 - (pbv * 2^24 * step + bmin)
                #   = (pbv * rec0 + q8) - rec1  [rec0 = -2^24*step, rec1 = bmin]
                u2 = wpool.tile([128, 8], F32, tag="u2")
                nc.vector.scalar_tensor_tensor(out=u2, in0=pbv,
                                               scalar=rec_sb[:, 0:1], in1=q8,
                                               op0=ALU.mult, op1=ALU.add)
                wr = wpool.tile([128, 8], F32, tag="wr")
                nc.vector.tensor_scalar(wr, u2, rec_sb[:, 1:2], None,
                                        op0=ALU.subtract)
                s = wpool.tile([128, 1], F32, tag="s")
                nc.vector.reduce_sum(s, wr.rearrange("p (o k) -> p o k", o=1),
                                     axis=AX.X)
                r = wpool.tile([128, 1], F32, tag="r")
                nc.vector.reciprocal(r, s)
                nc.vector.tensor_scalar(wt_all[:, i, :], wr, r[:, 0:1], 2.5,
                                        op0=ALU.mult, op1=ALU.mult)

            prev = None
            for i in range(n_tiles):
                fetch(i + 2)
                xh_sb, x88_sb = xtiles.pop(i)

                # corrections first: Pc = x8 @ wl8 + xl8 @ wh8 (DoubleRow
                # fp8, both at scale 2^16 relative to P0)
                pc = cppool.tile([128, E], F32)
                for hp in range(H_CHUNKS // 2):
                    sl = slice(2 * hp, 2 * hp + 2)
                    nc.tensor.matmul(pc, x88_sb[:, sl, 0:128], w88_sb[:, sl, 0:E],
                                     start=(hp == 0), stop=False,
                                     perf_mode=mybir.MatmulPerfMode.DoubleRow)
                for hp in range(H_CHUNKS // 2):
                    sl = slice(2 * hp, 2 * hp + 2)
                    nc.tensor.matmul(pc, x88_sb[:, sl, 128:256], w88_sb[:, sl, E:],
                                     start=False, stop=(hp == H_CHUNKS // 2 - 1),
                                     perf_mode=mybir.MatmulPerfMode.DoubleRow)
                # u1 = Pc * 2^-16 in fp16 (ACT), computed while the main
                # pass runs; PE then folds it into P0 with an identity
                # matmul so sigmoid can read a single PSUM tile.
                u1 = wpool.tile([128, E], F16, tag="u1")
                nc.scalar.activation(u1, pc, AF.Copy, scale=SCORR)

                # main pass: P0 = xh @ wh  (1024 * logit, fp16 operands)
                p0 = ppool.tile([128, E], F32)
                for ho in range(H_CHUNKS):
                    nc.tensor.matmul(p0, xh_sb[:, ho, :], wh_sb[:, ho, :],
                                     start=(ho == 0), stop=False)
                nc.tensor.matmul(p0, ident_sb, u1, start=False, stop=True)

                # stage A: scores = sigmoid(P0*2^-10); sfc = scores + bias.
                # Emitted ahead of the previous tile's post-chain so the
                # ACT->DVE->ACT ring never stalls the next tile's sigmoid.
                scores = wpool.tile([128, E], F32, tag="scores")
                nc.scalar.activation(scores, p0, AF.Sigmoid, scale=float(2.0 ** -10))
                sfc = wpool.tile([128, E], F32, tag="sfc")
                nc.gpsimd.tensor_add(sfc, scores, bias_sb)

                if prev is not None:
                    stage_b(*prev)
                prev = (i, sfc)

                if i == 13:
                    # input prefetch is done (fetch(15) just issued); SP is
                    # free from here, so these waits block nothing.
                    nc.sync.dma_start(idx_out[:12].rearrange("t p k -> p t k"),
                                      idx_u16[:, :12, :])
                    nc.sync.dma_start(wt_out[:12].rearrange("t p k -> p t k"),
                                      wt_all[:, :12, :])

            stage_b(*prev)
            nc.sync.dma_start(idx_out[12:].rearrange("t p k -> p t k"),
                              idx_u16[:, 12:, :])
            nc.sync.dma_start(wt_out[12:].rearrange("t p k -> p t k"),
                              wt_all[:, 12:, :])
    return nc


def prep_core_inputs(x_core, shared):
    n_tiles = x_core.shape[0] // 128
    x = np.ascontiguousarray(x_core, dtype=np.float32)
    xh = x.astype(np.float16)
    xl = x - xh.astype(np.float32)
    x8 = np.clip(x * np.float32(2.0 ** SX8), -240, 240).astype(NPF8)
    xl8 = np.clip(xl * np.float32(2.0 ** SXL), -240, 240).astype(NPF8)

    def tile_x(a):
        # [T, H] -> [n_tiles, 128p(h_inner), 16(h_outer), 128(t)]
        return np.ascontiguousarray(
            a.reshape(n_tiles, 128, H_CHUNKS, 128).transpose(0, 3, 2, 1))

    x88 = np.concatenate([tile_x(x8), tile_x(xl8)], axis=3)
    return {"xh": tile_x(xh), "x88": x88, **shared}


def prep_shared(weight, bias_vec):
    ws = np.ascontiguousarray(weight, dtype=np.float32) * 1024.0
    wh_ = ws.astype(np.float16)
    wl_ = ws - wh_.astype(np.float32)
    wl8 = np.clip(wl_ * np.float32(2.0 ** SWL), -240, 240).astype(NPF8)
    wh8 = np.clip(wh_.astype(np.float32) * np.float32(2.0 ** SWH),
                  -240, 240).astype(NPF8)

    def tile_w(a):
        # [E, H] -> [H, E] -> [128p(h_inner), 16(h_outer), E]
        return np.ascontiguousarray(a.T.reshape(H_CHUNKS, 128, E).transpose(1, 0, 2))

    w88 = np.concatenate([tile_w(wl8), tile_w(wh8)], axis=2)
    b = np.asarray(bias_vec, np.float32)
    bias_rep = np.broadcast_to(b, (128, E)).copy()

    # 4-bit packed bias: pbq in 0..15, quantum 2^-24 (stays below the 2^-19
    # ranking quantum so it never perturbs rank order beyond a tiebreak)
    bmin = np.float32(b.min())
    bmax = np.float32(b.max())
    step = np.float32((bmax - bmin) / 15.0) if bmax > bmin else np.float32(1.0)
    pbq = np.clip(np.round((b - bmin) / step), 0, 15).astype(np.float32)
    pb_rep = np.broadcast_to((pbq * np.float32(2.0 ** -24)).astype(np.float32),
                             (128, E)).copy()
    rec_const = np.broadcast_to(
        np.array([-np.float32(2.0 ** 24) * step, bmin], np.float32), (128, 2)).copy()
    return {"wh16": tile_w(wh_), "w88": w88, "bias_rep": bias_rep,
            "pb_rep": pb_rep, "rec_const": rec_const,
            "ident": np.eye(128, dtype=np.float16)}


_CACHED = {}


def _get_nc():
    if "nc" not in _CACHED:
        nc = bacc.Bacc("TRN2", num_devices=N_CORES)
        build_kernel(nc)
        nc.compile()
        _CACHED["nc"] = nc
    return _CACHED["nc"]


def make_in_maps(hidden_states, weight, e_score_correction_bias):
    x = np.asarray(hidden_states, np.float32).reshape(-1, H)
    shared = prep_shared(np.asarray(weight, np.float32),
                         np.asarray(e_score_correction_bias, np.float32))
    return [prep_core_inputs(x[c * T_CORE:(c + 1) * T_CORE], shared)
            for c in range(N_CORES)]


def kernel(hidden_states, weight, e_score_correction_bias):
    in_maps = make_in_maps(hidden_states, weight, e_score_correction_bias)
    nc = _get_nc()
    res = bass_utils.run_bass_kernel_spmd(nc, in_maps, core_ids=list(range(N_CORES)))
    idx = np.concatenate([r["idx_out"].reshape(-1, 8) for r in res.results], axis=0)
    wt = np.concatenate([r["wt_out"].reshape(-1, 8) for r in res.results], axis=0)
    return idx.astype(np.int32), wt.astype(np.float32)
